# revision 2
# baseline (speedup 1.0000x reference)
"""Bipartite GNN conv (variable->factor) Trainium2 kernel, v2.

8 NeuronCores, no collectives. Per core:
  - FACTORS range-sharded (6250/core, padded to 6272 = 49 windows x 128).
  - Host: per-core dedup of sender variable rows (~46.5k distinct ->
    2 int16 banks), receiver-sort edges, window = 128 consecutive local
    factors, slots per window padded to fixed caps (trailing -1 idx are
    trimmed free by the Q7 gather kernel).
  - Device per core/window:
      gvT [din, e]   <- dma_gather(transpose=True) of v_bf16 rows
                        (3 calls/window cycled over 4 SWDGE queues for
                        parallel Q7 descriptor gen)
      st  [e, t*128+s] = (iota == rrel[e,t])     one DVE op / window
      stT [s, t*128+e] = (rrelT == s_partition)  one DVE op / window
      per tile t: pm = stT_t.T @ A_blk + gvT_t.T @ W2   (PE, psum f32)
                  msb = relu(pm) bf16               (ACT/DVE alternate)
                  pagg[d,s] += msb.T @ st_t             (PE accum)
      out_blk = relu(aggT.T @ Wc2 + FT_blk.T @ Wc1 + bcomb)
  - A = F@W1+bmsg (bf16) and FT (factors^T, f32) live in SBUF.
"""

import os
import numpy as np

os.environ.setdefault("MYCRO_LOCAL_CACHE", "1")

D = 128
P = 128
NC = 8
WIN = 128
BANK = 32768
GCHUNK = 512

_LAST_EXEC_NS = None
_LAST_RES = None
_TRACE = bool(int(os.environ.get("GNN_KERNEL_TRACE", "0")))
_SKIP = set(os.environ.get("GNN_V2_SKIP", "").split(","))


def _install_profile_shim():
    import sys
    import types
    import ctypes
    import contextlib

    try:
        import antenv
        try:
            from antenv.axon_hooks import get_axon_ntff_profile_hook  # noqa
        except ImportError:
            mod = types.ModuleType("antenv.axon_hooks")
            mod._hook = None
            mod.set_axon_ntff_profile_hook = lambda h: setattr(mod, "_hook", h)
            mod.get_axon_ntff_profile_hook = lambda: mod._hook
            sys.modules["antenv.axon_hooks"] = mod
            antenv.axon_hooks = mod

        from antenv.axon_hooks import (  # noqa
            get_axon_ntff_profile_hook, set_axon_ntff_profile_hook)
        if get_axon_ntff_profile_hook() is None:
            lib = ctypes.CDLL("/opt/axon/libaxon_pjrt.so")
            if hasattr(lib, "axon_start_nrt_profile"):
                lib.axon_start_nrt_profile.argtypes = [
                    ctypes.POINTER(ctypes.c_int64), ctypes.c_size_t]
                lib.axon_start_nrt_profile.restype = ctypes.c_int64
                lib.axon_stop_nrt_profile.argtypes = [ctypes.c_char_p]
                lib.axon_stop_nrt_profile.restype = ctypes.c_int64

                @contextlib.contextmanager
                def _hook(output_dir, device_ids):
                    import jax
                    jax.devices()
                    if device_ids:
                        ids = (ctypes.c_int64 * len(device_ids))(*device_ids)
                        rc = lib.axon_start_nrt_profile(ids, len(device_ids))
                    else:
                        rc = lib.axon_start_nrt_profile(None, 0)
                    if rc != 0:
                        raise RuntimeError(f"start_nrt_profile rc={rc}")
                    try:
                        yield
                    finally:
                        n = lib.axon_stop_nrt_profile(str(output_dir).encode())
                        print(f"profile: {n} file(s) -> {output_dir}",
                              file=sys.stderr)

                set_axon_ntff_profile_hook(_hook)

        import concourse.bass_utils as bu
        bu.upload_artifacts = lambda tmpdir: f"local:{tmpdir}"
    except Exception as e:
        print(f"profile shim failed: {e}", file=sys.stderr)


def _wrap16(lin):
    """Linear idx list -> dma_gather layout [128, n/16] (16-part wrap, 8x)."""
    blk = lin.reshape(-1, 16).T.copy()
    return np.tile(blk, (8, 1))


def _pack_inputs(variables, factors, senders, receivers, W_msg, b_msg, W_comb,
                 b_comb, n_cores=NC, win=WIN):
    import ml_dtypes
    bf16 = ml_dtypes.bfloat16

    variables = np.ascontiguousarray(np.asarray(variables, dtype=np.float32))
    factors = np.ascontiguousarray(np.asarray(factors, dtype=np.float32))
    senders = np.asarray(senders).astype(np.int64)
    receivers = np.asarray(receivers).astype(np.int64)
    W_msg = np.asarray(W_msg, dtype=np.float32)
    b_msg = np.asarray(b_msg, dtype=np.float32).reshape(1, D)
    W_comb = np.asarray(W_comb, dtype=np.float32)
    b_comb = np.asarray(b_comb, dtype=np.float32).reshape(1, D)

    n_factors = factors.shape[0]
    f_loc = n_factors // n_cores
    assert f_loc * n_cores == n_factors
    nw = (f_loc + win - 1) // win
    f_pad = nw * win

    v_bf = variables.astype(bf16)

    order = np.argsort(receivers, kind="stable")
    rs = receivers[order]
    ss = senders[order]
    core_lo = np.searchsorted(rs, np.arange(n_cores) * f_loc)
    core_hi = np.searchsorted(rs, (np.arange(n_cores) + 1) * f_loc)

    # per-core dedup + remap of sender rows
    percore = []
    nv_list = []
    for c in range(n_cores):
        lo, hi = core_lo[c], core_hi[c]
        r_loc = (rs[lo:hi] - c * f_loc).astype(np.int64)
        s_gl = ss[lo:hi]
        uniq, inv = np.unique(s_gl, return_inverse=True)
        nv_list.append(len(uniq))
        percore.append((r_loc, inv, uniq))
    nv_max = max(nv_list)
    nv_pad = ((nv_max + 127) // 128) * 128
    assert nv_pad <= 2 * BANK

    # per (core, window, bank) counts -> global caps
    counts = np.zeros((n_cores, nw, 2), np.int64)
    for c in range(n_cores):
        r_loc, s_loc, _ = percore[c]
        w_of = r_loc // win
        b_of = s_loc // BANK
        np.add.at(counts[c], (w_of, b_of), 1)
    cap = counts.max(axis=(0, 1))
    Cb = (((cap + GCHUNK - 1) // GCHUNK) * GCHUNK).astype(np.int64)
    Cb = np.maximum(Cb, GCHUNK)
    C = int(Cb.sum())
    K = C // P
    b_off = np.array([0, Cb[0]])

    # gather call chunks: split bank chunks at GCHUNK boundaries
    chunks = []  # (col_off, n, bank)
    for b in range(2):
        off = 0
        while off < Cb[b]:
            n = min(GCHUNK, Cb[b] - off)
            chunks.append((int(b_off[b] + off), int(n), b))
            off += n

    iota_rep = np.tile(np.arange(P, dtype=np.int16), (P, K))
    iota_col = np.arange(P, dtype=np.float32).reshape(P, 1)

    in_maps = []
    for c in range(n_cores):
        r_loc, s_loc, uniq = percore[c]
        w_of = r_loc // win
        b_of = s_loc // BANK
        ordwb = np.lexsort((r_loc, b_of, w_of))
        r_loc, s_loc, w_of, b_of = (r_loc[ordwb], s_loc[ordwb], w_of[ordwb],
                                    b_of[ordwb])
        cnt = counts[c]
        cum = np.zeros((nw, 2), np.int64)
        cum.flat[1:] = np.cumsum(cnt.flat)[:-1]
        j = np.arange(len(r_loc)) - cum[w_of, b_of]   # rank within (w, b)
        slot = b_off[b_of] + j                        # column slot in window

        # vidx: [nw, C] int16 bank-relative, pads gather row 0 (the
        # one-hot matrices zero out pad contributions)
        vidx = np.zeros((nw, C), np.int16)
        vidx[w_of, slot] = (s_loc - b_of * BANK).astype(np.int16)
        vidx_w = np.concatenate([_wrap16(vidx[w]) for w in range(nw)], axis=0)

        # rrel: [nw*P, K] int16; rrel[w*P+p, t] = r of slot (t,p), pad -1
        rrel = np.full((nw, P, K), -1, np.int16)
        t_of = slot // P
        p_of = slot % P
        rrel[w_of, p_of, t_of] = (r_loc - w_of * win).astype(np.int16)
        # rrelT: [nw*P, K*P] int16; col t*128+e = r of slot (t,e), all parts
        rrelT = np.repeat(
            rrel.transpose(0, 2, 1).reshape(nw, 1, K * P), P,
            axis=1).astype(np.float32)

        floc = np.zeros((f_pad, D), np.float32)
        floc[:f_loc] = factors[c * f_loc:(c + 1) * f_loc]

        vtab = np.zeros((nv_pad, D), bf16)
        vtab[:len(uniq)] = v_bf[uniq]

        im = {
            "vtab": vtab,
            "factors_loc": floc,
            "vidx": vidx_w,
            "rrel": rrel.reshape(nw * P, K),
            "rrelT": rrelT.reshape(nw * P, K * P),
            "W1": np.ascontiguousarray(W_msg[:D]),
            "W2b": np.ascontiguousarray(W_msg[D:]).astype(bf16),
            "Wc1": np.ascontiguousarray(W_comb[:D]),
            "Wc2": np.ascontiguousarray(W_comb[D:]),
            "bmsg": b_msg, "bcomb": b_comb,
            "ones_r": np.ones((1, D), np.float32),
            "ident": np.eye(P, dtype=np.float32),
            "iota_rep": iota_rep,
            "iota_col": iota_col,
        }
        in_maps.append(im)

    params = dict(f_loc=f_loc, f_pad=f_pad, nw=nw, K=K, C=C,
                  nv_pad=int(nv_pad), chunks=chunks, n_cores=n_cores)
    return in_maps, params


def _build_nc(params):
    import concourse.bacc as bacc
    import concourse.tile as tile
    import concourse.mybir as mybir
    from concourse import library_config

    f32 = mybir.dt.float32
    bf16 = mybir.dt.bfloat16
    i16 = mybir.dt.int16
    nw, K, C = params["nw"], params["K"], params["C"]
    f_pad, nv_pad = params["f_pad"], params["nv_pad"]
    chunks = params["chunks"]
    relu_fn = mybir.ActivationFunctionType.Relu

    nc = bacc.Bacc("TRN2", target_bir_lowering=False, debug=False,
                   num_swdge_queues=4)

    t_vtab = nc.dram_tensor("vtab", [nv_pad, D], bf16, kind="ExternalInput")
    t_floc = nc.dram_tensor("factors_loc", [f_pad, D], f32,
                            kind="ExternalInput")
    t_vidx = nc.dram_tensor("vidx", [nw * P, C // 16], i16,
                            kind="ExternalInput")
    t_rrel = nc.dram_tensor("rrel", [nw * P, K], i16, kind="ExternalInput")
    t_rrelT = nc.dram_tensor("rrelT", [nw * P, K * P], f32,
                             kind="ExternalInput")
    t_W1 = nc.dram_tensor("W1", [D, D], f32, kind="ExternalInput")
    t_W2b = nc.dram_tensor("W2b", [D, D], bf16, kind="ExternalInput")
    t_Wc1 = nc.dram_tensor("Wc1", [D, D], f32, kind="ExternalInput")
    t_Wc2 = nc.dram_tensor("Wc2", [D, D], f32, kind="ExternalInput")
    t_bmsg = nc.dram_tensor("bmsg", [1, D], f32, kind="ExternalInput")
    t_bcomb = nc.dram_tensor("bcomb", [1, D], f32, kind="ExternalInput")
    t_ones = nc.dram_tensor("ones_r", [1, D], f32, kind="ExternalInput")
    t_id = nc.dram_tensor("ident", [P, P], f32, kind="ExternalInput")
    t_iota = nc.dram_tensor("iota_rep", [P, K * P], i16,
                            kind="ExternalInput")
    t_iotac = nc.dram_tensor("iota_col", [P, 1], f32, kind="ExternalInput")
    t_out = nc.dram_tensor("out", [f_pad, D], f32, kind="ExternalOutput")

    qn = [0]

    def next_q():
        q = qn[0]
        qn[0] = (qn[0] + 1) % 4
        return q

    with tile.TileContext(nc) as tc:
        with (
            tc.tile_pool(name="const", bufs=1) as cpool,
            tc.tile_pool(name="ft", bufs=1) as ftpool,
            tc.tile_pool(name="io", bufs=3) as iopool,
            tc.tile_pool(name="gv", bufs=2) as gvpool,
            tc.tile_pool(name="vix", bufs=8) as vixpool,
            tc.tile_pool(name="st", bufs=2) as stpool,
            tc.tile_pool(name="work", bufs=4) as wpool,
            tc.tile_pool(name="ps_t", bufs=1, space="PSUM") as ps_t,
            tc.tile_pool(name="ps_m", bufs=2, space="PSUM") as ps_m,
            tc.tile_pool(name="ps_agg", bufs=1, space="PSUM") as ps_agg,
        ):
            nc.gpsimd.load_library(library_config.mlp)

            def cload(t, shape, dt):
                s = cpool.tile(shape, dt, tag=t.name)
                nc.sync.dma_start(out=s[:], in_=t[:])
                return s

            W1 = cload(t_W1, [D, D], f32)
            W2b = cload(t_W2b, [D, D], bf16)
            Wc1 = cload(t_Wc1, [D, D], f32)
            Wc2 = cload(t_Wc2, [D, D], f32)
            bmsg = cload(t_bmsg, [1, D], f32)
            bcomb = cload(t_bcomb, [1, D], f32)
            ones_r = cload(t_ones, [1, D], f32)
            ident = cload(t_id, [P, P], f32)
            iota = cload(t_iota, [P, K * P], i16)
            iotac = cload(t_iotac, [P, 1], f32)

            FT = ftpool.tile([P, f_pad], f32, tag="FT")
            A_sb = ftpool.tile([P, f_pad], bf16, tag="A_sb")

            # zero gather buffers once (pads are trailing-trimmed -> stale)
            gv_bufs = []
            for i in range(2):
                gv_buf = gvpool.tile([P, C], bf16, tag=f"gv{i}")
                gv_bufs.append(gv_buf)
            for g in gv_bufs:
                nc.vector.memset(g[:], 0.0)

            # ---- precompute FT + A (bf16, SBUF)
            if "pre" in _SKIP:
                nc.vector.memset(FT[:], 0.0)
                nc.vector.memset(A_sb[:], 0.0)
            for blk in range(0 if "pre" in _SKIP else nw):
                cols = slice(blk * P, (blk + 1) * P)
                fl = iopool.tile([P, D], f32, tag="fload")
                nc.sync.dma_start(out=fl[:], in_=t_floc[cols, :])
                pt = ps_t.tile([P, P], f32, tag="pt")
                nc.tensor.transpose(out=pt[:], in_=fl[:], identity=ident[:])
                nc.vector.tensor_copy(out=FT[:, cols], in_=pt[:])
                pa = ps_t.tile([P, D], f32, tag="pa")
                nc.tensor.matmul(pa[:], lhsT=ones_r[:1, :], rhs=bmsg[:1, :],
                                 start=True, stop=False)
                nc.tensor.matmul(pa[:], lhsT=FT[:, cols], rhs=W1[:],
                                 start=False, stop=True)
                nc.scalar.copy(out=A_sb[:, cols], in_=pa[:])

            # ---- edge phase
            for w in range(nw):
                wrow = slice(w * P, (w + 1) * P)
                wcols = slice(w * P, (w + 1) * P)

                vix = vixpool.tile([P, C // 16], i16, tag="vix")
                nc.sync.dma_start(out=vix[:], in_=t_vidx[wrow, :])
                rr = iopool.tile([P, K], i16, tag="rr")
                nc.sync.dma_start(out=rr[:], in_=t_rrel[wrow, :])
                rrT = iopool.tile([P, K * P], f32, tag="rrT")
                nc.sync.dma_start(out=rrT[:], in_=t_rrelT[wrow, :])

                gvT = gv_bufs[w % 2]
                for (coff, n, b) in ([] if "gather" in _SKIP else chunks):
                    lo = b * BANK
                    hi = min((b + 1) * BANK, nv_pad)
                    nc.gpsimd.dma_gather(
                        out_ap=gvT[:, coff:coff + n].rearrange(
                            "p (c n) -> p c n", c=1),
                        in_ap=t_vtab[lo:hi, :],
                        idxs_ap=vix[:, coff // 16:(coff + n) // 16],
                        num_idxs=n, num_idxs_reg=n,
                        elem_size=D, transpose=True,
                        queue_num=next_q())

                st_all = stpool.tile([P, K * P], bf16, tag=f"st{w % 2}")
                if "st" in _SKIP:
                    nc.vector.memset(st_all[:], 0.0)
                else:
                    nc.vector.scalar_tensor_tensor(
                        out=st_all[:].rearrange("p (k s) -> p k s", k=K),
                        in0=iota[:].rearrange("p (k s) -> p k s", k=K),
                        scalar=0,
                        in1=rr[:, :, None].broadcast_to([P, K, P]),
                        op0=mybir.AluOpType.add,
                        op1=mybir.AluOpType.is_equal)
                stT_all = stpool.tile([P, K * P], bf16, tag=f"stT{w % 2}")
                if "stT" in _SKIP:
                    nc.vector.memset(stT_all[:], 0.0)
                else:
                    nc.vector.tensor_scalar(
                        out=stT_all[:], in0=rrT[:], scalar1=iotac[:],
                        scalar2=None, op0=mybir.AluOpType.is_equal)

                pagg = ps_agg.tile([P, P], f32, tag=f"pagg{w % 2}")
                for t in range(0 if "mm" in _SKIP else K):
                    tcols = slice(t * P, (t + 1) * P)
                    pm = ps_m.tile([P, D], f32)
                    nc.tensor.matmul(pm[:], lhsT=stT_all[:, tcols],
                                     rhs=A_sb[:, wcols], start=True,
                                     stop=False)
                    nc.tensor.matmul(pm[:], lhsT=gvT[:, tcols], rhs=W2b[:],
                                     start=False, stop=True)
                    msb = wpool.tile([P, D], bf16, tag="msb")
                    if t % 2 == 0:
                        nc.scalar.activation(msb[:], pm[:], relu_fn)
                    else:
                        nc.vector.tensor_scalar(
                            out=msb[:], in0=pm[:], scalar1=0.0, scalar2=None,
                            op0=mybir.AluOpType.max)
                    nc.tensor.matmul(pagg[:], lhsT=msb[:],
                                     rhs=st_all[:, tcols],
                                     start=(t == 0), stop=(t == K - 1))

                aggT = wpool.tile([P, P], f32, tag="aggT")
                if "mm" in _SKIP or "out" in _SKIP:
                    nc.vector.memset(aggT[:], 0.0)
                else:
                    nc.vector.tensor_copy(out=aggT[:], in_=pagg[:])
                po = ps_t.tile([P, D], f32, tag="po")
                if "out" not in _SKIP:
                    nc.tensor.matmul(po[:], lhsT=ones_r[:1, :],
                                     rhs=bcomb[:1, :], start=True, stop=False)
                    nc.tensor.matmul(po[:], lhsT=aggT[:], rhs=Wc2[:],
                                     start=False, stop=False)
                    nc.tensor.matmul(po[:], lhsT=FT[:, wcols], rhs=Wc1[:],
                                     start=False, stop=True)
                osb = iopool.tile([P, D], f32, tag="osb")
                if "out" in _SKIP:
                    nc.vector.memset(osb[:], 0.0)
                else:
                    nc.scalar.activation(osb[:], po[:], relu_fn)
                nc.sync.dma_start(out=t_out[wrow, :], in_=osb[:])

    nc.compile()
    return nc


def kernel(**inputs):
    global _LAST_EXEC_NS, _LAST_RES
    from concourse.bass_utils import run_bass_kernel_spmd

    in_maps, params = _pack_inputs(**inputs)
    n_cores = params["n_cores"]
    ncores_env = int(os.environ.get("GNN_V2_CORES", "0"))
    if ncores_env:
        n_cores = ncores_env
        in_maps = in_maps[:n_cores]
    nc = _build_nc(params)
    if _TRACE:
        _install_profile_shim()
        try:
            res = run_bass_kernel_spmd(nc, in_maps, list(range(n_cores)),
                                       trace=True, tmpdir=os.environ.get(
                                           "GNN_KERNEL_TRACE_DIR"))
        except Exception as e:
            import sys
            print(f"traced run failed ({e}); retrying untraced",
                  file=sys.stderr)
            res = run_bass_kernel_spmd(nc, in_maps, list(range(n_cores)))
    else:
        res = run_bass_kernel_spmd(nc, in_maps, list(range(n_cores)))
    _LAST_EXEC_NS = res.exec_time_ns
    _LAST_RES = res
    f_loc = params["f_loc"]
    out = np.concatenate([res.results[c]["out"][:f_loc]
                          for c in range(n_cores)], axis=0)
    return out.astype(np.float32)
